# revision 57
# baseline (speedup 1.0000x reference)
"""Trainium2 Bass kernel for nn_KernelDensityLoss (KDE softmax loss).

Math: the reference's O(B^2*D) pairwise log-prob matrix collapses to
per-class sufficient statistics.  For row i and class c,

  A[i,c] = M*sq[i] + Ssq[c] - 2*G[i,c]     (G = X @ S^T, sq = ||x_i||^2,
                                            S_c = class sum, Ssq_c = class
                                            sum of squared norms)
  P[i,c] = -0.5*A[i,c] / (var*m_c)         (m_c = M-1 own class, M else)
  loss   = sum_i relu(logsumexp_c P[i,c] - P[i,own])

The Gaussian normalisation constant cancels in logsumexp - own.  With
c0 = -0.5/(var*M), the kernel computes q[i,c] = c0*(Ssq_c - 2*G[i,c]);
the per-row M*sq[i] term is dropped (a per-row additive constant K
cancels in z_c = P_c - (M/(M-1))*P_own except for a -c0*K/(M-1)
residue that folds into the per-row exp bias):

  z_c = q_c + b2_i,   b2_i = -(M/(M-1))*own_q_i - c0*M*sq_i/(M-1)
  se  = sum_c exp(z_c), own column's term replaced by its exact value
        1 via se += 1 - exp(z_own);   L_i = relu(ln(se)).

Distribution: phase 3 (per-row losses) is data-parallel over the 8
cores (896 rows each).  The tiny class stats are computed REDUNDANTLY
on every core from the full batch: on this runtime a cross-core
collective costs ~60us end-to-end (host-mediated trigger + rank-start
skew), far more than the matmul it saves, so no collective is used.

Stats: the full batch streams once as bf16 moving data against a
per-class one-hot stationary (labels are class-sorted, so each
1024-row class spans 8 aligned 128-row tiles -> one stationary per
class chunk), giving S = class sums.  Ssq comes from fp32 row norms
(squares on vector/scalar, per-tile reduce on gpsimd) column-summed by
a single ones-stationary matmul - no squares matmul.

All big inputs are host-pre-tiled to [128, *] so every DMA is
per-partition contiguous (multi-KB descriptors, near line rate).  The
combined exp+ln activation table set is loaded once up front; all Exp
inputs are pre-biased and batched into ONE [128, 7*7] Exp, so no table
swap sits on the critical path.
NOTE: tensor_tensor_reduce crashes this runtime (scalar_tensor_tensor
+accum_out replaces it); a TensorTensor with two PSUM operands fails
the walrus verifier (evacuate first).
"""

import numpy as np

import concourse.bass as bass
import concourse.bacc as bacc
import concourse.mybir as mybir
import concourse.tile as tile
from concourse.bass_utils import run_bass_kernel_spmd

B = 7168      # total rows
C = 7         # classes
M = 1024      # rows per class
D = 256       # embedding dim
NCORES = 8
R = B // NCORES          # 896 rows per core
T = R // 128             # 7 row-tiles of 128 per core
TF = B // 128            # 56 tiles over the full batch
HC = TF // C             # 8 tiles per class chunk

F32 = mybir.dt.float32
BF16 = mybir.dt.bfloat16
AX = mybir.AxisListType
AF = mybir.ActivationFunctionType
ALU = mybir.AluOpType

PKW = 4 + C + T * C + C * C   # packed fp32 input width: consts|ident|y|ycls
NLE_SET = 6  # act_info.json index of natural_log_exp_and_others


def build_program():
    nc = bacc.Bacc(
        "TRN2",
        target_bir_lowering=False,
        debug=False,
        enable_asserts=True,
        num_devices=NCORES,
    )

    xf_d = nc.dram_tensor("xf", [128, TF * D], BF16, kind="ExternalInput")
    xbig_d = nc.dram_tensor("xbig", [128, T * D + 2 * R], BF16,
                            kind="ExternalInput")
    pk_d = nc.dram_tensor("pk", [128, PKW], F32, kind="ExternalInput")
    out_d = nc.dram_tensor("loss_part", [128, T], F32, kind="ExternalOutput")

    with tile.TileContext(nc) as tc:
        with (
            tc.tile_pool(name="persist", bufs=1) as pp,
            tc.tile_pool(name="sqscratch", bufs=2) as pq,
            tc.tile_pool(name="chunk", bufs=2) as pc,
        ):
            # ---- persistent tiles ----
            xfb = pp.tile([128, TF, D], BF16, tag="xfb")   # full batch
            xsb = pp.tile([128, TF, D], BF16, tag="xsb")   # its squares
            xbig = pp.tile([128, T * D + 2 * R], BF16, tag="xbig")
            pk = pp.tile([128, PKW], F32, tag="pk")
            ycls = pp.tile([128, C, C], BF16, tag="ycls")  # class one-hot bcast
            sq = pp.tile([128, T], F32, tag="sq")          # own ||x_i||^2
            b_t2 = pp.tile([128, T], F32, tag="b_t2")      # sq*M*c0/(M-1)
            sA = pp.tile([7, 512], F32, tag="sA")
            st7 = pp.tile([7, 256], F32, tag="st7")        # S [c, d]
            ssq71 = pp.tile([7, 1], F32, tag="ssq71")      # per-class Ssq col
            sqA = pp.tile([128, C], F32, tag="sqA")        # Ssq partial (acc)
            ssqA_row = pp.tile([1, C], F32, tag="ssqA_row")
            ones_row = pp.tile([1, 128], F32, tag="ones_row")
            s2d = pp.tile([7, C], F32, tag="s2d")          # diag(Ssq)
            ones7 = pp.tile([7, 128], F32, tag="ones7")
            shsc = pp.tile([128, 2 * C], BF16, tag="shsc")  # -2*c0*S^T
            ssqb = pp.tile([128, C], F32, tag="ssqb")      # c0*Ssq broadcast
            ones_col = pp.tile([128, 1], F32, tag="ones_col")
            q_all = pp.tile([128, T, C], F32, tag="q_all")
            zq_all = pp.tile([128, T, C], F32, tag="zq_all")
            e_all = pp.tile([128, T, C], F32, tag="e_all")
            own_all = pp.tile([128, T], F32, tag="own_all")
            b2_all = pp.tile([128, T], F32, tag="b2_all")
            zo_all = pp.tile([128, T], F32, tag="zo_all")
            eo_all = pp.tile([128, T], F32, tag="eo_all")
            se_all = pp.tile([128, T], F32, tag="se_all")
            se_fix = pp.tile([128, T], F32, tag="se_fix")
            lnse = pp.tile([128, T], F32, tag="lnse")
            accL = pp.tile([128, T], F32, tag="accL")
            accT = pp.tile([128, 1], F32, tag="accT")
            out_s = pp.tile([1, 1], F32, tag="out_s")

            # views into the packed fp32 input
            consts = pk[:, 0:4]
            ident = pk[0:C, 4:4 + C]
            ytile = pk[:, 4 + C:4 + C + T * C].rearrange(
                "p (t c) -> p t c", c=C)
            yclsf = pk[:, 4 + C + T * C:PKW]
            # views into the packed bf16 own-shard input
            xb = xbig[:, 0:T * D].rearrange("p (t d) -> p t d", d=D)
            xt0 = xbig[:, T * D:T * D + R]
            xt1 = xbig[:, T * D + R:T * D + 2 * R]

            # ---- loads (all per-partition contiguous) ----
            # pk + xbig triggers ride the scalar sequencer so the sync
            # sequencer's serial ~700ns DIRECT2D dispatches all go to the
            # xf chunk stream that gates the stats matmuls.
            nc.scalar.dma_start(out=pk[:], in_=pk_d[:, :])
            nc.scalar.dma_start(out=xbig[:], in_=xbig_d[:, :])
            for j in range(C):
                nc.sync.dma_start(
                    out=xfb[:, HC * j:HC * j + HC, :],
                    in_=xf_d[:, HC * j * D:(HC * j + HC) * D].rearrange(
                        "p (a d) -> p a d", d=D))

            # one activation-table load (exp+ln+square+copy set), early but
            # after the DMA triggers so it does not delay them
            nc.scalar.add_instruction(mybir.InstLoadActFuncSet(
                name=nc.get_next_instruction_name(), act_func_set_id=NLE_SET))

            nc.vector.tensor_copy(ycls[:].rearrange("p a c -> p (a c)"),
                                  yclsf)
            nc.gpsimd.memset(ones_col[:], 1.0)
            nc.gpsimd.memset(ones7[:], 1.0)
            nc.vector.memset(ones_row[:], 1.0)


            # ---- own-shard row norms (vector; xbig-gated, runs during
            # the stats stream since vector has no chunk work) ----
            for t in range(T):
                scr = pq.tile([128, D], F32, tag="sqscr")
                nc.vector.tensor_mul(scr[:], xb[:, t, :], xb[:, t, :])
                nc.vector.reduce_sum(sq[:, t:t + 1], scr[:], axis=AX.X)
            nc.vector.tensor_scalar_mul(b_t2[:], sq[:], consts[:, 2:3])

            # ---- full-batch stats, one class chunk (8 tiles) at a time.
            # S matmuls for chunk j are gated only by chunk j's DMA; the S2
            # matmuls (over the on-the-fly squares) are emitted with a
            # 2-chunk lag so their squares are long done -- the PE stream
            # never stalls and fills the DMA-paced idle gaps. ----
            LAG = 0
            with tc.tile_pool(name="psum_stat", bufs=1, space="PSUM") as qstat:
                psA = qstat.tile([7, 512], F32, tag="psA")
                psB = qstat.tile([7, 512], F32, tag="psB")

                def s2_mms(j):
                    g = HC * j
                    nc.tensor.matmul(
                        psB[:], lhsT=ycls[:, j, :],
                        rhs=xsb[:, g + 6:g + 8, :],
                        start=(j == 0), stop=(j == C - 1))

                for j in range(C):
                    g = HC * j
                    # Ssq partial for tiles 0-3: one Square with free accum
                    # (a chunk is one class, so the row-sum over 4 tiles IS
                    # the needed partial) -- no squares matmul for them
                    scr6 = pq.tile([128, 6, D], BF16, tag="scr6")
                    nc.scalar.activation(scr6[:], xfb[:, g:g + 6, :],
                                         AF.Square, bias=0.0, scale=1.0,
                                         accum_out=sqA[:, j:j + 1])
                    # tiles 6-7 keep the matmul path (squares materialized)
                    nc.gpsimd.tensor_mul(xsb[:, g + 6:g + 8, :],
                                         xfb[:, g + 6:g + 8, :],
                                         xfb[:, g + 6:g + 8, :])
                    y_j = ycls[:, j, :]
                    for v in range(HC // 2):
                        nc.tensor.matmul(psA[:], lhsT=y_j,
                                         rhs=xfb[:, g + 2 * v:g + 2 * v + 2, :],
                                         start=(j == 0 and v == 0),
                                         stop=(j == C - 1 and v == HC // 2 - 1))
                    if j >= LAG:
                        s2_mms(j - LAG)
                # psA is complete here; evacuate + fold + reduce psB
                # while still inside the 2-bank stats scope
                nc.vector.tensor_copy(sA[:], psA[:])
                nc.vector.tensor_add(st7[:], sA[:, 0:256], sA[:, 256:512])
                nc.vector.reduce_sum(ssq71[:], psB[:], axis=AX.X)

            # ---- S^T transposes (own 2-bank scope) ----
            with tc.tile_pool(name="psum_t", bufs=2, space="PSUM") as qt:
                for hh in range(2):
                    tp = qt.tile([128, C], F32, tag="tp")
                    nc.tensor.transpose(tp[:],
                                        st7[:, 128 * hh:128 * hh + 128],
                                        ident)
                    # shsc = -2*c0 * S^T bf16 (phase-3 moving operand)
                    nc.vector.tensor_scalar_mul(shsc[:, C * hh:C * hh + C],
                                                tp[:], consts[:, 1:2])

            # ---- phase 3 (8 banks: 7 pP + shared Ssq tile).  The G
            # matmuls and own-column extraction are emitted BEFORE the
            # Ssq finalize so the PE never stalls waiting for the scalar
            # accum stream (sqA) to finish. ----
            with tc.tile_pool(name="psum_p", bufs=1, space="PSUM") as qp:
                pPs = [qp.tile([128, C], F32, tag=f"pP{u}", name=f"pP{u}")
                       for u in range(T)]
                for u in range(T):
                    lo, hi = u * 128, (u + 1) * 128
                    nc.tensor.matmul(pPs[u][:], lhsT=xt0[:, lo:hi],
                                     rhs=shsc[:, 0:C], start=True, stop=False)
                    nc.tensor.matmul(pPs[u][:], lhsT=xt1[:, lo:hi],
                                     rhs=shsc[:, C:2 * C], start=False,
                                     stop=True)

                # Ssq finalize: one shared PSUM tile; MM-sa waits on the
                # scalar accum stream, but only the zq stage needs ssqb
                nc.vector.tensor_scalar_mul(s2d[:], ident, ssq71[:, 0:1])
                ps_x = qp.tile([128, C], F32, tag="ps_x")
                nc.tensor.matmul(ps_x[0:1, :], lhsT=ones_col[:], rhs=sqA[:],
                                 start=True, stop=True)
                nc.vector.tensor_copy(ssqA_row[:], ps_x[0:1, :])
                nc.tensor.matmul(ps_x[:], lhsT=ones7[:], rhs=s2d[:],
                                 start=True, stop=False)
                nc.tensor.matmul(ps_x[:], lhsT=ones_row[:], rhs=ssqA_row[:],
                                 start=False, stop=True)
                nc.vector.tensor_scalar_mul(ssqb[:], ps_x[:], consts[:, 0:1])

                for u in range(T):
                    # q = c0*(Ssq - 2G); pP is PSUM, so vector only
                    nc.vector.tensor_add(q_all[:, u, :], pPs[u][:], ssqb[:])
                    # own_q = sum_c mask*q (mask-mult with free row sum)
                    scr7 = pc.tile([128, C], F32, tag="scr7")
                    nc.vector.scalar_tensor_tensor(
                        out=scr7[:], in0=q_all[:, u, :], scalar=1.0,
                        in1=ytile[:, u, :],
                        op0=ALU.mult, op1=ALU.mult,
                        accum_out=own_all[:, u:u + 1],
                    )

                # b2 = -(M/(M-1))*own_q - b_t2 ;  zo = -(1/(M-1))*own_q - b_t2
                nc.vector.scalar_tensor_tensor(
                    out=b2_all[:], in0=own_all[:], scalar=-float(M) / (M - 1),
                    in1=b_t2[:], op0=ALU.mult, op1=ALU.subtract,
                )
                nc.vector.scalar_tensor_tensor(
                    out=zo_all[:], in0=own_all[:], scalar=-1.0 / (M - 1),
                    in1=b_t2[:], op0=ALU.mult, op1=ALU.subtract,
                )

                # z = q + b2 (bias broadcast per tile), then ONE batched Exp
                for u in range(T):
                    eng = nc.vector if u % 2 == 0 else nc.gpsimd
                    eng.tensor_scalar_add(zq_all[:, u, :], q_all[:, u, :],
                                          b2_all[:, u:u + 1])
                nc.scalar.activation(e_all[:], zq_all[:], AF.Exp)
                nc.scalar.activation(eo_all[:], zo_all[:], AF.Exp)
                # se per tile (innermost-C reduction), then own-column fix
                nc.vector.reduce_sum(
                    se_all[:].rearrange("p (t o) -> p t o", o=1),
                    e_all[:], axis=AX.X)
                nc.vector.scalar_tensor_tensor(
                    out=se_fix[:], in0=se_all[:], scalar=1.0, in1=eo_all[:],
                    op0=ALU.add, op1=ALU.subtract,
                )
                nc.scalar.activation(lnse[:], se_fix[:], AF.Ln)
                # relu on the same queue as Ln (no cross-engine handoff);
                # out-DMA triggered from the long-idle sync sequencer
                nc.scalar.activation(accL[:], lnse[:], AF.Relu,
                                     bias=0.0, scale=1.0)
                # per-row relu'd losses go out as-is; the host sums the
                # 8 x [128, T] partials (it already sums the 8 cores)
                nc.sync.dma_start(out=out_d[:, :], in_=accL[:])

    nc.compile()
    return nc


_NC_CACHE = None


def _get_nc():
    global _NC_CACHE
    if _NC_CACHE is None:
        _NC_CACHE = build_program()
    return _NC_CACHE


def make_in_maps(embeddings, variance):
    import ml_dtypes

    X = np.ascontiguousarray(np.asarray(embeddings, dtype=np.float32))
    assert X.shape == (B, D), X.shape
    var = float(np.asarray(variance))

    labels = np.repeat(np.arange(C), M)  # reference ignores `target`
    Yfull = np.zeros((B, C), np.float32)
    Yfull[np.arange(B), labels] = 1.0

    c0 = -0.5 / (var * M)

    Xb = X.astype(ml_dtypes.bfloat16)
    # pre-tiled full batch: xf_t[p, a*D+d] = X[a*128+p, d]
    xf_t = np.ascontiguousarray(
        Xb.reshape(TF, 128, D).transpose(1, 0, 2).reshape(128, TF * D))

    in_maps = []
    for k in range(NCORES):
        s = slice(k * R, (k + 1) * R)
        Xs = Xb[s]
        xb_t = Xs.reshape(T, 128, D).transpose(1, 0, 2).reshape(128, T * D)
        xt = Xs.T  # [D, R]
        xbig = np.concatenate([xb_t, xt[0:128, :], xt[128:256, :]], axis=1)

        Ys = Yfull[s]  # [R, C]
        y_t = Ys.reshape(T, 128, C).transpose(1, 0, 2).reshape(128, T * C)
        pk = np.zeros((128, PKW), np.float32)
        pk[:, 0] = c0
        pk[:, 1] = -2.0 * c0
        pk[:, 2] = M * c0 / (M - 1)
        pk[0:C, 4:4 + C] = np.eye(C, dtype=np.float32)
        pk[:, 4 + C:4 + C + T * C] = y_t
        ycls = np.zeros((128, C * C), np.float32)
        for c in range(C):
            ycls[:, c * C + c] = 1.0
        pk[:, 4 + C + T * C:PKW] = ycls

        in_maps.append({
            "xf": xf_t,
            "xbig": np.ascontiguousarray(xbig),
            "pk": pk,
        })
    return in_maps


def kernel(embeddings, target, variance):
    del target  # labels are balanced & class-sorted by construction (as in reference)
    nc = _get_nc()
    in_maps = make_in_maps(embeddings, variance)
    res = run_bass_kernel_spmd(nc, in_maps, list(range(NCORES)))
    total = 0.0
    for k in range(NCORES):
        total += float(np.asarray(res.results[k]["loss_part"], np.float64).sum())
    return np.float32(total)
